# revision 18
# baseline (speedup 1.0000x reference)
import numpy as np

# Mamba net, hardcoded dims (see problem): B=128, L=28, F=28, DM=256,
# DI=512, DS=16, DR=16, K=3, NL=5, OUT=10.  8-core data parallel over B.
NL = 5
NCORES = 8
BL = 16            # batch per core
L = 28             # seq len
N = BL * L         # 448 tokens per core, b-major t-minor
F = 28
DM = 256
DI = 512
DS = 16
DR = 16
K = 3
OUT = 10
NCH = DI // 128    # 4 chunks of d_inner
EX = BL * DS * L   # 7168 expanded free size (b, s, t)

_CACHE = {}


def _build(dve_chunks=(0, 1, 2, 3)):
    """Build the bass program. dve_chunks: chunk ids whose elementwise
    expanded work runs on DVE (rest currently also DVE; knob reserved)."""
    import concourse.bacc as bacc
    import concourse.bass as bass
    import concourse.mybir as mybir
    import concourse.tile as tile
    from contextlib import ExitStack

    f32 = mybir.dt.float32
    bf16 = mybir.dt.bfloat16
    Alu = mybir.AluOpType
    Act = mybir.ActivationFunctionType
    ts = bass.ts

    nc = bacc.Bacc("TRN2", target_bir_lowering=False, debug=False,
                   enable_asserts=False)

    xT_d = nc.dram_tensor("xT", [F, N], f32, kind="ExternalInput").ap()
    ipw_d = nc.dram_tensor("ipwT", [F, DM], f32, kind="ExternalInput").ap()
    win_d = nc.dram_tensor("winT", [NL, 2, 128, 2 * DI], f32, kind="ExternalInput").ap()
    xw_d = nc.dram_tensor("xwT", [NL, NCH, 128, 64], f32, kind="ExternalInput").ap()
    dtw_d = nc.dram_tensor("dtwT", [NL, DR, DI], f32, kind="ExternalInput").ap()
    ow_d = nc.dram_tensor("owT", [NL, NCH, 128, DM], f32, kind="ExternalInput").ap()
    cwv_d = nc.dram_tensor("cwv", [NL, 128, NCH, K], f32, kind="ExternalInput").ap()
    vec_d = nc.dram_tensor("vec", [NL, 128, NCH, 3], f32, kind="ExternalInput").ap()
    cls_d = nc.dram_tensor("clsT", [2, 128, OUT], f32, kind="ExternalInput").ap()
    out_d = nc.dram_tensor("out", [OUT, BL], f32, kind="ExternalOutput").ap()
    # DRAM scratch for cross-partition broadcast of B/C (2 alternating)
    bc_scr = [nc.dram_tensor(f"bc_scr{i}", [2 * DS, N], bf16).ap() for i in range(2)]

    with tile.TileContext(nc) as tc, ExitStack() as ctx:
        cpool = ctx.enter_context(tc.tile_pool(name="const", bufs=1))
        wpool = ctx.enter_context(tc.tile_pool(name="weights", bufs=2))
        hpool = ctx.enter_context(tc.tile_pool(name="h", bufs=4))
        apool = ctx.enter_context(tc.tile_pool(name="act", bufs=1))
        tpool = ctx.enter_context(tc.tile_pool(name="trans", bufs=8))
        bcpool = ctx.enter_context(tc.tile_pool(name="bc", bufs=1))
        bigpool = ctx.enter_context(tc.tile_pool(name="big", bufs=2))
        psum = ctx.enter_context(tc.tile_pool(name="ps", bufs=8, space="PSUM"))

        def ptile(p, nm="ps"):
            return psum.tile([p, N], f32, padded_shape=[p, 512], name=nm,
                             tag="ps")

        # ---- load constants
        xT = cpool.tile([F, N], f32, tag="xT")
        nc.sync.dma_start(xT, xT_d)
        ipw = cpool.tile([F, DM], f32, tag="ipw")
        nc.sync.dma_start(ipw, ipw_d)
        cls_t = cpool.tile([128, 2 * OUT], f32, tag="cls")
        cls_v = cls_t.rearrange("p (k o) -> p k o", k=2)
        nc.sync.dma_start(cls_v, cls_d.transpose([1, 0, 2]))

        # ---- input projection: h[m] = ipw[:, m*128:...].T @ xT
        h_cur = []
        for m in range(2):
            ps = ptile(128)
            nc.tensor.matmul(ps, ipw[:, ts(m, 128)], xT, start=True, stop=True)
            h0 = hpool.tile([128, N], f32, tag="h")
            nc.scalar.copy(h0, ps)
            h_cur.append(h0)

        for l in range(NL):
            # ---- per-layer weights
            wt = wpool.tile([128, 2 * 2 * DI], f32, tag="win")
            wt_v = wt.rearrange("p (k j) -> p k j", k=2)
            nc.sync.dma_start(wt_v, win_d[l].transpose([1, 0, 2]))
            xwt = wpool.tile([128, NCH * 64], f32, tag="xw")
            xwt_v = xwt.rearrange("p (c r) -> p c r", c=NCH)
            nc.sync.dma_start(xwt_v, xw_d[l].transpose([1, 0, 2]))
            dtwt = wpool.tile([DR, DI], f32, tag="dtw")
            nc.sync.dma_start(dtwt, dtw_d[l])
            owt = wpool.tile([128, NCH * DM], f32, tag="ow")
            owt_v = owt.rearrange("p (c m) -> p c m", c=NCH)
            nc.sync.dma_start(owt_v, ow_d[l].transpose([1, 0, 2]))
            cwt = wpool.tile([128, NCH * K], f32, tag="cw")
            cwt_v = cwt.rearrange("p (c k) -> p c k", c=NCH)
            nc.sync.dma_start(cwt_v, cwv_d[l])
            vt = wpool.tile([128, NCH * 3], f32, tag="vec")
            vt_v = vt.rearrange("p (c k) -> p c k", c=NCH)
            nc.sync.dma_start(vt_v, vec_d[l])

            # ---- in_proj: xz[j] = in_w[j*128:...] @ h  (j<4: xb, j>=4: z)
            xz = []
            for j in range(8):
                ps = ptile(128)
                for k in range(2):
                    nc.tensor.matmul(ps, wt_v[:, k, ts(j, 128)], h_cur[k],
                                     start=(k == 0), stop=(k == 1))
                xz.append(ps)

            # ---- conv + silu(u), silu(z)
            u = apool.tile([128, NCH * N], f32, tag="u")
            u_v = u.rearrange("p (c n) -> p c n", c=NCH)
            sz = apool.tile([128, NCH * N], f32, tag="sz")
            sz_v = sz.rearrange("p (c n) -> p c n", c=NCH)
            for c in range(NCH):
                xb = tpool.tile([128, N], f32, tag="t")
                nc.scalar.copy(xb, xz[c])
                xc = tpool.tile([128, N], f32, tag="t")
                nc.vector.tensor_scalar(xc, xb, cwt_v[:, c, 2:3],
                                        vt_v[:, c, 0:1], Alu.mult, Alu.add)
                xb3 = xb.rearrange("p (b t) -> p b t", b=BL)
                xc3 = xc.rearrange("p (b t) -> p b t", b=BL)
                nc.vector.scalar_tensor_tensor(
                    xc3[:, :, 1:], xb3[:, :, :L - 1], cwt_v[:, c, 1:2],
                    xc3[:, :, 1:], Alu.mult, Alu.add)
                nc.vector.scalar_tensor_tensor(
                    xc3[:, :, 2:], xb3[:, :, :L - 2], cwt_v[:, c, 0:1],
                    xc3[:, :, 2:], Alu.mult, Alu.add)
                nc.scalar.activation(u_v[:, c, :], xc, Act.Silu)
                nc.scalar.activation(sz_v[:, c, :], xz[4 + c], Act.Silu)

            # ---- x_proj: dbc = xw @ u   (64 x N; rows 0:16 dt, 32:64 B,C;
            # rows 16:32 zero-padded so B,C start on a partition quadrant)
            dbc = psum.tile([64, N], f32, padded_shape=[64, 512], tag="ps")
            for c in range(NCH):
                nc.tensor.matmul(dbc, xwt_v[:, c, :], u_v[:, c, :],
                                 start=(c == 0), stop=(c == NCH - 1))
            dt_sb = tpool.tile([DR, N], f32, tag="dt", bufs=2)
            nc.scalar.copy(dt_sb, dbc[0:DR, :])
            bc_sb = tpool.tile([2 * DS, N], bf16, tag="bc", bufs=2)
            nc.scalar.copy(bc_sb, dbc[32:64, :])

            # ---- broadcast B,C to all 128 partitions via DRAM roundtrip
            # expanded layout: free = (s, b, t), t innermost for the scan
            scr = bc_scr[l % 2]
            nc.sync.dma_start(scr, bc_sb)
            Brep = bcpool.tile([128, EX], bf16, tag="Brep")
            Crep = bcpool.tile([128, EX], bf16, tag="Crep")
            srcB = scr[0:DS, :].unsqueeze(0).broadcast_to([128, DS, N])
            srcC = scr[DS:2 * DS, :].unsqueeze(0).broadcast_to([128, DS, N])
            nc.sync.dma_start(Brep.rearrange("p (s n) -> p s n", s=DS), srcB)
            nc.sync.dma_start(Crep.rearrange("p (s n) -> p s n", s=DS), srcC)
            Brep4 = Brep.rearrange("p (s b t) -> p s b t", s=DS, b=BL)

            # ---- out_proj accumulators
            op_ps = [ptile(128) for _ in range(2)]

            for c in range(NCH):
                # delta = softplus(x) = ln(1 + exp(x)),  x = dtw @ dt + dtb
                # (no softplus ACT table on cayman; Exp+Ln share one table)
                dtp = ptile(128)
                nc.tensor.matmul(dtp, dtwt[:, ts(c, 128)], dt_sb,
                                 start=True, stop=True)
                ex = tpool.tile([128, N], f32, tag="t")
                nc.scalar.activation(ex, dtp, Act.Exp, bias=vt_v[:, c, 1:2])
                nc.gpsimd.tensor_single_scalar(ex, ex, 1.0, Alu.add)
                delta = tpool.tile([128, N], f32, tag="t")
                nc.scalar.activation(delta, ex, Act.Ln)
                # Dpow[:, s, b, t] = exp(-delta)^(s+1)  (A[d,s] = -(s+1))
                Dp = bigpool.tile([128, EX], bf16, tag="Dp")
                Dp4 = Dp.rearrange("p (s b t) -> p s b t", s=DS, b=BL)
                d3 = delta.rearrange("p (b t) -> p b t", b=BL)
                nc.scalar.activation(Dp4[:, 0, :, :], d3, Act.Exp, scale=-1.0)
                nc.vector.tensor_mul(Dp4[:, 1, :, :], Dp4[:, 0, :, :],
                                     Dp4[:, 0, :, :])
                nc.vector.tensor_mul(
                    Dp4[:, 2:4, :, :], Dp4[:, 0:2, :, :],
                    Dp4[:, 1:2, :, :].broadcast_to([128, 2, BL, L]))
                nc.vector.tensor_mul(
                    Dp4[:, 4:8, :, :], Dp4[:, 0:4, :, :],
                    Dp4[:, 3:4, :, :].broadcast_to([128, 4, BL, L]))
                nc.vector.tensor_mul(
                    Dp4[:, 8:16, :, :], Dp4[:, 0:8, :, :],
                    Dp4[:, 7:8, :, :].broadcast_to([128, 8, BL, L]))
                # reset state at sequence starts
                nc.vector.memset(Dp4[:, :, :, 0:1], 0.0)
                # inj = (delta*u) * B
                du = tpool.tile([128, N], bf16, tag="du")
                nc.gpsimd.tensor_mul(du, delta, u_v[:, c, :])
                du4 = (du.rearrange("p (b t) -> p b t", b=BL).unsqueeze(1)
                       .broadcast_to([128, DS, BL, L]))
                inj = bigpool.tile([128, EX], bf16, tag="inj")
                inj4 = inj.rearrange("p (s b t) -> p s b t", s=DS, b=BL)
                nc.vector.tensor_mul(inj4, du4, Brep4)
                # scan: hs[t] = Dp[t]*hs[t-1] + inj[t]
                hs = bigpool.tile([128, EX], bf16, tag="hs")
                nc.vector.tensor_tensor_scan(hs, Dp, inj, 0.0,
                                             Alu.mult, Alu.add)
                # y = sum_s hs * C
                nc.gpsimd.tensor_mul(hs, hs, Crep)
                h4 = hs.rearrange("p (s b t) -> p s b t", s=DS, b=BL)
                nc.gpsimd.tensor_add(h4[:, 0:8, :, :], h4[:, 0:8, :, :],
                                     h4[:, 8:16, :, :])
                nc.gpsimd.tensor_add(h4[:, 0:4, :, :], h4[:, 0:4, :, :],
                                     h4[:, 4:8, :, :])
                nc.gpsimd.tensor_add(h4[:, 0:2, :, :], h4[:, 0:2, :, :],
                                     h4[:, 2:4, :, :])
                ysum = tpool.tile([128, N], f32, tag="t")
                y3 = ysum.rearrange("p (b t) -> p b t", b=BL)
                nc.gpsimd.tensor_add(y3, h4[:, 0, :, :], h4[:, 1, :, :])
                # yg = (u*D + ysum) * silu(z)
                yg = tpool.tile([128, N], f32, tag="t")
                nc.vector.scalar_tensor_tensor(yg, u_v[:, c, :],
                                               vt_v[:, c, 2:3], ysum,
                                               Alu.mult, Alu.add)
                nc.vector.tensor_mul(yg, yg, sz_v[:, c, :])
                # out_proj accumulate
                for m in range(2):
                    nc.tensor.matmul(op_ps[m], owt_v[:, c, ts(m, 128)], yg,
                                     start=(c == 0), stop=(c == NCH - 1))

            # ---- residual
            h_new = []
            for m in range(2):
                hn = hpool.tile([128, N], f32, tag="h")
                nc.vector.tensor_add(hn, h_cur[m], op_ps[m])
                h_new.append(hn)
            h_cur = h_new

        # ---- classifier (mean over t folded into weights)
        lg = psum.tile([OUT, N], f32, padded_shape=[OUT, 512], tag="ps")
        for k in range(2):
            nc.tensor.matmul(lg, cls_v[:, k, :], h_cur[k],
                             start=(k == 0), stop=(k == 1))
        lgm = cpool.tile([OUT, BL], f32, tag="lgm")
        nc.vector.tensor_reduce(lgm, lg.rearrange("p (b t) -> p b t", b=BL),
                                axis=mybir.AxisListType.X, op=Alu.add)
        nc.sync.dma_start(out_d, lgm)

    nc.compile()
    return nc


def _prep_weights(inputs):
    f = np.float32
    ipwT = np.ascontiguousarray(inputs["input_proj_w"].T, dtype=f)
    winT = np.ascontiguousarray(
        inputs["in_proj_w"].transpose(0, 2, 1).reshape(NL, 2, 128, 2 * DI), dtype=f)
    xw_pad = np.zeros((NL, 64, DI), f)
    xw_pad[:, 0:DR] = inputs["x_proj_w"][:, 0:DR]
    xw_pad[:, 32:64] = inputs["x_proj_w"][:, DR:DR + 2 * DS]
    xwT = np.ascontiguousarray(
        xw_pad.transpose(0, 2, 1).reshape(NL, NCH, 128, 64), dtype=f)
    dtwT = np.ascontiguousarray(inputs["dt_proj_w"].transpose(0, 2, 1), dtype=f)
    owT = np.ascontiguousarray(
        inputs["out_proj_w"].transpose(0, 2, 1).reshape(NL, NCH, 128, DM), dtype=f)
    cwv = np.ascontiguousarray(
        inputs["conv_w"].reshape(NL, NCH, 128, K).transpose(0, 2, 1, 3), dtype=f)
    vec = np.stack([inputs["conv_b"].reshape(NL, NCH, 128),
                    inputs["dt_proj_b"].reshape(NL, NCH, 128),
                    inputs["D"].reshape(NL, NCH, 128)], axis=-1)
    vec = np.ascontiguousarray(vec.transpose(0, 2, 1, 3), dtype=f)
    clsT = np.ascontiguousarray(
        (inputs["classifier_w"].T / np.float32(L)).reshape(2, 128, OUT), dtype=f)
    return {"ipwT": ipwT, "winT": winT, "xwT": xwT, "dtwT": dtwT,
            "owT": owT, "cwv": cwv, "vec": vec, "clsT": clsT}


def _run(inputs, trace=False):
    from concourse.bass_utils import run_bass_kernel_spmd
    if "nc" not in _CACHE:
        _CACHE["nc"] = _build()
    nc = _CACHE["nc"]
    w = _prep_weights(inputs)
    x = np.asarray(inputs["x"], dtype=np.float32)
    in_maps = []
    for i in range(NCORES):
        xs = x[i * BL:(i + 1) * BL, 0]                 # (16, 28, 28) b,t,f
        xT = np.ascontiguousarray(xs.transpose(2, 0, 1).reshape(F, N))
        m = {"xT": xT}
        m.update(w)
        in_maps.append(m)
    res = run_bass_kernel_spmd(nc, in_maps, list(range(NCORES)), trace=trace)
    parts = [res.results[i]["out"].T for i in range(NCORES)]   # (16, 10) each
    out = np.ascontiguousarray(np.concatenate(parts, axis=0), dtype=np.float32)
    return out, res


def kernel(**inputs) -> np.ndarray:
    out, _ = _run(inputs, trace=False)
    return out


# revision 19
# speedup vs baseline: 1.2662x; 1.2662x over previous
import numpy as np

# Mamba net, hardcoded dims (see problem): B=128, L=28, F=28, DM=256,
# DI=512, DS=16, DR=16, K=3, NL=5, OUT=10.  8-core data parallel over B.
NL = 5
NCORES = 8
BL = 16            # batch per core
L = 28             # seq len
N = BL * L         # 448 tokens per core, b-major t-minor
F = 28
DM = 256
DI = 512
DS = 16
DR = 16
K = 3
OUT = 10
NCH = DI // 128    # 4 chunks of d_inner
EX = BL * DS * L   # 7168 expanded free size (s, b, t)
LP = L + K - 1     # 30, zero-padded time for conv-as-matmul

_CACHE = {}


def _build():
    import concourse.bacc as bacc
    import concourse.bass as bass
    import concourse.mybir as mybir
    import concourse.tile as tile
    from contextlib import ExitStack

    f32 = mybir.dt.float32
    bf16 = mybir.dt.bfloat16
    Alu = mybir.AluOpType
    Act = mybir.ActivationFunctionType
    ts = bass.ts

    nc = bacc.Bacc("TRN2", target_bir_lowering=False, debug=False,
                   enable_asserts=False)

    xT_d = nc.dram_tensor("xT", [F, N], f32, kind="ExternalInput").ap()
    ipw_d = nc.dram_tensor("ipwT", [F, DM], f32, kind="ExternalInput").ap()
    # in_proj xb-half folded with conv: 6 = K taps x 2 DM-halves
    wxb_d = nc.dram_tensor("wxbT", [NL, K * 2, 128, DI], bf16,
                           kind="ExternalInput").ap()
    wz_d = nc.dram_tensor("wzT", [NL, 2, 128, DI], bf16,
                          kind="ExternalInput").ap()
    xw_d = nc.dram_tensor("xwT", [NL, NCH, 128, 64], bf16,
                          kind="ExternalInput").ap()
    dtw_d = nc.dram_tensor("dtwT", [NL, DR, DI], bf16,
                           kind="ExternalInput").ap()
    ow_d = nc.dram_tensor("owT", [NL, NCH, 128, DM], bf16,
                          kind="ExternalInput").ap()
    vec_d = nc.dram_tensor("vec", [NL, 128, NCH, 3], f32,
                           kind="ExternalInput").ap()
    cls_d = nc.dram_tensor("clsT", [2, 128, OUT], f32, kind="ExternalInput").ap()
    out_d = nc.dram_tensor("out", [OUT, BL], f32, kind="ExternalOutput").ap()
    # DRAM scratch for cross-partition broadcast of B/C (2 alternating)
    bc_scr = [nc.dram_tensor(f"bc_scr{i}", [2 * DS, N], bf16).ap()
              for i in range(2)]

    with tile.TileContext(nc) as tc, ExitStack() as ctx:
        cpool = ctx.enter_context(tc.tile_pool(name="const", bufs=1))
        wpool = ctx.enter_context(tc.tile_pool(name="weights", bufs=2))
        hpool = ctx.enter_context(tc.tile_pool(name="h", bufs=4))
        hbpool = ctx.enter_context(tc.tile_pool(name="hb", bufs=2))
        apool = ctx.enter_context(tc.tile_pool(name="act", bufs=1))
        tpool = ctx.enter_context(tc.tile_pool(name="trans", bufs=8))
        bcpool = ctx.enter_context(tc.tile_pool(name="bc", bufs=1))
        bigpool = ctx.enter_context(tc.tile_pool(name="big", bufs=2))
        psum = ctx.enter_context(tc.tile_pool(name="ps", bufs=8, space="PSUM"))

        def ptile(p, nm="ps"):
            return psum.tile([p, N], f32, padded_shape=[p, 512], name=nm,
                             tag="ps")

        # ---- load constants
        xT = cpool.tile([F, N], f32, tag="xT")
        nc.sync.dma_start(xT, xT_d)
        ipw = cpool.tile([F, DM], f32, tag="ipw")
        nc.sync.dma_start(ipw, ipw_d)
        cls_t = cpool.tile([128, 2 * OUT], f32, tag="cls")
        cls_v = cls_t.rearrange("p (k o) -> p k o", k=2)
        nc.sync.dma_start(cls_v, cls_d.transpose([1, 0, 2]))

        # ---- input projection: h[m] = ipw[:, m*128:...].T @ xT
        h_cur = []
        for m in range(2):
            ps = ptile(128)
            nc.tensor.matmul(ps, ipw[:, ts(m, 128)], xT, start=True, stop=True)
            h0 = hpool.tile([128, N], f32, tag="h")
            nc.scalar.copy(h0, ps)
            h_cur.append(h0)

        for l in range(NL):
            # ---- per-layer weights
            wxb = wpool.tile([128, K * 2 * DI], bf16, tag="wxb")
            wxb_v = wxb.rearrange("p (q j) -> p q j", q=K * 2)
            nc.sync.dma_start(wxb_v, wxb_d[l].transpose([1, 0, 2]))
            wz = wpool.tile([128, 2 * DI], bf16, tag="wz")
            wz_v = wz.rearrange("p (m j) -> p m j", m=2)
            nc.sync.dma_start(wz_v, wz_d[l].transpose([1, 0, 2]))
            xwt = wpool.tile([128, NCH * 64], bf16, tag="xw")
            xwt_v = xwt.rearrange("p (c r) -> p c r", c=NCH)
            nc.sync.dma_start(xwt_v, xw_d[l].transpose([1, 0, 2]))
            dtwt = wpool.tile([DR, DI], bf16, tag="dtw")
            nc.sync.dma_start(dtwt, dtw_d[l])
            owt = wpool.tile([128, NCH * DM], bf16, tag="ow")
            owt_v = owt.rearrange("p (c m) -> p c m", c=NCH)
            nc.sync.dma_start(owt_v, ow_d[l].transpose([1, 0, 2]))
            vt = wpool.tile([128, NCH * 3], f32, tag="vec")
            vt_v = vt.rearrange("p (c k) -> p c k", c=NCH)
            nc.sync.dma_start(vt_v, vec_d[l])

            # ---- bf16 zero-padded h for in_proj (+folded conv taps)
            hp3 = []
            for m in range(2):
                hp = hbpool.tile([128, BL * LP], bf16, tag="hp")
                v = hp.rearrange("p (b t) -> p b t", b=BL)
                nc.vector.memset(v[:, :, 0:K - 1], 0.0)
                nc.scalar.copy(v[:, :, K - 1:],
                               h_cur[m].rearrange("p (b t) -> p b t", b=BL))
                hp3.append(v)

            # ---- in_proj: xb-half with conv folded in (3 prescaled taps),
            # z-half plain.  xc = sum_k (cw_k*Wxb) @ h[t-2+k]
            xz = []
            for j in range(NCH):
                ps = ptile(128)
                ps3 = ps.rearrange("p (b t) -> p b t", b=BL)
                mm = 0
                for k in range(K):
                    for m in range(2):
                        nc.tensor.matmul(ps3, wxb_v[:, k * 2 + m, ts(j, 128)],
                                         hp3[m][:, :, k:k + L],
                                         start=(mm == 0), stop=(mm == 5))
                        mm += 1
                xz.append(ps)
            for j in range(NCH):
                ps = ptile(128)
                ps3 = ps.rearrange("p (b t) -> p b t", b=BL)
                for m in range(2):
                    nc.tensor.matmul(ps3, wz_v[:, m, ts(j, 128)],
                                     hp3[m][:, :, K - 1:],
                                     start=(m == 0), stop=(m == 1))
                xz.append(ps)

            # ---- u = silu(xc + conv_b), sz = silu(z)   (all Silu-table)
            u = apool.tile([128, NCH * N], bf16, tag="u")
            u_v = u.rearrange("p (c n) -> p c n", c=NCH)
            sz = apool.tile([128, NCH * N], bf16, tag="sz")
            sz_v = sz.rearrange("p (c n) -> p c n", c=NCH)
            for c in range(NCH):
                nc.scalar.activation(u_v[:, c, :], xz[c], Act.Silu,
                                     bias=vt_v[:, c, 0:1])
                nc.scalar.activation(sz_v[:, c, :], xz[NCH + c], Act.Silu)

            # ---- x_proj: dbc = xw @ u   (64 x N; rows 0:16 dt, 32:64 B,C;
            # rows 16:32 zero-padded so B,C start on a partition quadrant)
            dbc = psum.tile([64, N], f32, padded_shape=[64, 512], tag="ps")
            for c in range(NCH):
                nc.tensor.matmul(dbc, xwt_v[:, c, :], u_v[:, c, :],
                                 start=(c == 0), stop=(c == NCH - 1))
            dt_sb = tpool.tile([DR, N], bf16, tag="dt", bufs=2)
            nc.scalar.copy(dt_sb, dbc[0:DR, :])
            bc_sb = tpool.tile([2 * DS, N], bf16, tag="bc", bufs=2)
            nc.scalar.copy(bc_sb, dbc[32:64, :])

            # ---- broadcast B,C to all 128 partitions via DRAM roundtrip
            # expanded layout: free = (s, b, t), t innermost for the scan
            scr = bc_scr[l % 2]
            nc.sync.dma_start(scr, bc_sb)
            Brep = bcpool.tile([128, EX], bf16, tag="Brep")
            Crep = bcpool.tile([128, EX], bf16, tag="Crep")
            srcB = scr[0:DS, :].unsqueeze(0).broadcast_to([128, DS, N])
            srcC = scr[DS:2 * DS, :].unsqueeze(0).broadcast_to([128, DS, N])
            nc.sync.dma_start(Brep.rearrange("p (s n) -> p s n", s=DS), srcB)
            nc.sync.dma_start(Crep.rearrange("p (s n) -> p s n", s=DS), srcC)
            Brep4 = Brep.rearrange("p (s b t) -> p s b t", s=DS, b=BL)

            # ---- out_proj accumulators
            op_ps = [ptile(128) for _ in range(2)]

            for c in range(NCH):
                # delta = softplus(x) = ln(1 + exp(x)),  x = dtw @ dt + dtb
                # (no softplus ACT table on cayman; Exp+Ln share one table)
                dtp = ptile(128)
                nc.tensor.matmul(dtp, dtwt[:, ts(c, 128)], dt_sb,
                                 start=True, stop=True)
                ex = tpool.tile([128, N], f32, tag="t")
                nc.scalar.activation(ex, dtp, Act.Exp, bias=vt_v[:, c, 1:2])
                delta = tpool.tile([128, N], f32, tag="t")
                nc.scalar.activation(delta, ex, Act.Ln, bias=1.0)
                # Dpow[:, s, b, t] = exp(-(s+1)*delta)  (A[d,s] = -(s+1))
                Dp = bigpool.tile([128, EX], bf16, tag="Dp")
                Dp4 = Dp.rearrange("p (s b t) -> p s b t", s=DS, b=BL)
                for s in range(DS):
                    nc.scalar.activation(Dp[:, ts(s, N)], delta, Act.Exp,
                                         scale=-(s + 1.0))
                # reset state at sequence starts
                nc.vector.memset(Dp4[:, :, :, 0:1], 0.0)
                # inj = (delta*u) * B
                du = tpool.tile([128, N], bf16, tag="du")
                nc.gpsimd.tensor_mul(du, delta, u_v[:, c, :])
                du4 = (du.rearrange("p (b t) -> p b t", b=BL).unsqueeze(1)
                       .broadcast_to([128, DS, BL, L]))
                inj = bigpool.tile([128, EX], bf16, tag="inj")
                inj4 = inj.rearrange("p (s b t) -> p s b t", s=DS, b=BL)
                nc.vector.tensor_mul(inj4, du4, Brep4)
                # scan: hs[t] = Dp[t]*hs[t-1] + inj[t]
                hs = bigpool.tile([128, EX], bf16, tag="hs")
                nc.vector.tensor_tensor_scan(hs, Dp, inj, 0.0,
                                             Alu.mult, Alu.add)
                # y = sum_s hs * C  (Cmul on Pool, tree L1 on DVE, rest Pool)
                nc.gpsimd.tensor_mul(hs, hs, Crep)
                h4 = hs.rearrange("p (s b t) -> p s b t", s=DS, b=BL)
                nc.vector.tensor_add(h4[:, 0:8, :, :], h4[:, 0:8, :, :],
                                     h4[:, 8:16, :, :])
                nc.gpsimd.tensor_add(h4[:, 0:4, :, :], h4[:, 0:4, :, :],
                                     h4[:, 4:8, :, :])
                nc.gpsimd.tensor_add(h4[:, 0:2, :, :], h4[:, 0:2, :, :],
                                     h4[:, 2:4, :, :])
                ysum = tpool.tile([128, N], f32, tag="t")
                y3 = ysum.rearrange("p (b t) -> p b t", b=BL)
                nc.gpsimd.tensor_add(y3, h4[:, 0, :, :], h4[:, 1, :, :])
                # yg = (u*D + ysum) * silu(z)
                yg = tpool.tile([128, N], bf16, tag="t")
                nc.vector.scalar_tensor_tensor(yg, u_v[:, c, :],
                                               vt_v[:, c, 2:3], ysum,
                                               Alu.mult, Alu.add)
                nc.gpsimd.tensor_mul(yg, yg, sz_v[:, c, :])
                # out_proj accumulate
                for m in range(2):
                    nc.tensor.matmul(op_ps[m], owt_v[:, c, ts(m, 128)], yg,
                                     start=(c == 0), stop=(c == NCH - 1))

            # ---- residual
            h_new = []
            for m in range(2):
                hn = hpool.tile([128, N], f32, tag="h")
                nc.vector.tensor_add(hn, h_cur[m], op_ps[m])
                h_new.append(hn)
            h_cur = h_new

        # ---- classifier (mean over t folded into weights)
        lg = psum.tile([OUT, N], f32, padded_shape=[OUT, 512], tag="ps")
        for k in range(2):
            nc.tensor.matmul(lg, cls_v[:, k, :], h_cur[k],
                             start=(k == 0), stop=(k == 1))
        lgm = cpool.tile([OUT, BL], f32, tag="lgm")
        nc.vector.tensor_reduce(lgm, lg.rearrange("p (b t) -> p b t", b=BL),
                                axis=mybir.AxisListType.X, op=Alu.add)
        nc.sync.dma_start(out_d, lgm)

    nc.compile()
    return nc


def _prep_weights(inputs):
    from ml_dtypes import bfloat16 as bf
    f = np.float32
    ipwT = np.ascontiguousarray(inputs["input_proj_w"].T, dtype=f)
    inw = np.asarray(inputs["in_proj_w"], dtype=f)        # (NL, 1024, 256)
    cw = np.asarray(inputs["conv_w"], dtype=f)            # (NL, 512, 3)
    # tap k prescaled: Wk[d, m] = cw[d, k] * Wxb[d, m]
    wxb = inw[:, None, :DI, :] * cw.transpose(0, 2, 1)[:, :, :, None]
    wxbT = np.ascontiguousarray(
        wxb.transpose(0, 1, 3, 2).reshape(NL, K, 2, 128, DI)
        .reshape(NL, K * 2, 128, DI), dtype=bf)
    wzT = np.ascontiguousarray(
        inw[:, DI:, :].transpose(0, 2, 1).reshape(NL, 2, 128, DI), dtype=bf)
    xw_pad = np.zeros((NL, 64, DI), f)
    xw_pad[:, 0:DR] = inputs["x_proj_w"][:, 0:DR]
    xw_pad[:, 32:64] = inputs["x_proj_w"][:, DR:DR + 2 * DS]
    xwT = np.ascontiguousarray(
        xw_pad.transpose(0, 2, 1).reshape(NL, NCH, 128, 64), dtype=bf)
    dtwT = np.ascontiguousarray(
        np.asarray(inputs["dt_proj_w"]).transpose(0, 2, 1), dtype=bf)
    owT = np.ascontiguousarray(
        np.asarray(inputs["out_proj_w"]).transpose(0, 2, 1)
        .reshape(NL, NCH, 128, DM), dtype=bf)
    vec = np.stack([np.asarray(inputs["conv_b"]).reshape(NL, NCH, 128),
                    np.asarray(inputs["dt_proj_b"]).reshape(NL, NCH, 128),
                    np.asarray(inputs["D"]).reshape(NL, NCH, 128)], axis=-1)
    vec = np.ascontiguousarray(vec.transpose(0, 2, 1, 3), dtype=f)
    clsT = np.ascontiguousarray(
        (np.asarray(inputs["classifier_w"]).T / np.float32(L))
        .reshape(2, 128, OUT), dtype=f)
    return {"ipwT": ipwT, "wxbT": wxbT, "wzT": wzT, "xwT": xwT, "dtwT": dtwT,
            "owT": owT, "vec": vec, "clsT": clsT}


def _run(inputs, trace=False):
    from concourse.bass_utils import run_bass_kernel_spmd
    if "nc" not in _CACHE:
        _CACHE["nc"] = _build()
    nc = _CACHE["nc"]
    w = _prep_weights(inputs)
    x = np.asarray(inputs["x"], dtype=np.float32)
    in_maps = []
    for i in range(NCORES):
        xs = x[i * BL:(i + 1) * BL, 0]                 # (16, 28, 28) b,t,f
        xT = np.ascontiguousarray(xs.transpose(2, 0, 1).reshape(F, N))
        m = {"xT": xT}
        m.update(w)
        in_maps.append(m)
    res = run_bass_kernel_spmd(nc, in_maps, list(range(NCORES)), trace=trace)
    parts = [res.results[i]["out"].T for i in range(NCORES)]   # (16, 10) each
    out = np.ascontiguousarray(np.concatenate(parts, axis=0), dtype=np.float32)
    return out, res


def kernel(**inputs) -> np.ndarray:
    out, _ = _run(inputs, trace=False)
    return out


# revision 23
# speedup vs baseline: 1.4060x; 1.1105x over previous
import numpy as np

# Mamba net, hardcoded dims (see problem): B=128, L=28, F=28, DM=256,
# DI=512, DS=16, DR=16, K=3, NL=5, OUT=10.  8-core data parallel over B.
NL = 5
NCORES = 8
BL = 16            # batch per core
L = 28             # seq len
N = BL * L         # 448 tokens per core, b-major t-minor
F = 28
DM = 256
DI = 512
DS = 16
DR = 16
K = 3
OUT = 10
NCH = DI // 128    # 4 chunks of d_inner
EX = BL * DS * L   # 7168 expanded free size (s, b, t)
LP = L + K - 1     # 30, zero-padded time for conv-as-matmul

_CACHE = {}


def _build():
    import concourse.bacc as bacc
    import concourse.bass as bass
    import concourse.mybir as mybir
    import concourse.tile as tile
    import concourse.hw_specs as hw_specs
    from contextlib import ExitStack

    # Route Exp and Ln to the shared natural_log_exp_and_others ACT table
    # (the greedy table pass otherwise alternates exp_and_others /
    # natural_log, reloading tables between every Exp<->Ln transition).
    # Table indices are preserved; only fn membership is masked.
    if not getattr(hw_specs, "_explog_patched", False):
        _orig = hw_specs.get_activation_tables

        def _patched(arch):
            t = dict(_orig(arch))
            if "natural_log_exp_and_others" not in t:
                return t
            E = mybir.ActivationFunctionType.Exp
            Ln = mybir.ActivationFunctionType.Ln
            return {k: (v if k == "natural_log_exp_and_others"
                        else v - {E, Ln}) for k, v in t.items()}

        hw_specs.get_activation_tables = _patched
        hw_specs._explog_patched = True
        if getattr(bacc, "get_activation_tables", None) is not None:
            bacc.get_activation_tables = _patched

    f32 = mybir.dt.float32
    bf16 = mybir.dt.bfloat16
    Alu = mybir.AluOpType
    Act = mybir.ActivationFunctionType
    ts = bass.ts

    nc = bacc.Bacc("TRN2", target_bir_lowering=False, debug=False,
                   enable_asserts=False)

    xT_d = nc.dram_tensor("xT", [F, N], f32, kind="ExternalInput").ap()
    ipw_d = nc.dram_tensor("ipwT", [F, DM], f32, kind="ExternalInput").ap()
    # in_proj xb-half folded with conv: 6 = K taps x 2 DM-halves
    wxb_d = nc.dram_tensor("wxbT", [NL, K * 2, 128, DI], bf16,
                           kind="ExternalInput").ap()
    wz_d = nc.dram_tensor("wzT", [NL, 2, 128, DI], bf16,
                          kind="ExternalInput").ap()
    xw_d = nc.dram_tensor("xwT", [NL, NCH, 128, 64], bf16,
                          kind="ExternalInput").ap()
    dtw_d = nc.dram_tensor("dtwT", [NL, DR, DI], bf16,
                           kind="ExternalInput").ap()
    ow_d = nc.dram_tensor("owT", [NL, NCH, 128, DM], bf16,
                          kind="ExternalInput").ap()
    vec_d = nc.dram_tensor("vec", [NL, 128, NCH, 3], f32,
                           kind="ExternalInput").ap()
    cls_d = nc.dram_tensor("clsT", [2, 128, OUT], f32, kind="ExternalInput").ap()
    out_d = nc.dram_tensor("out", [OUT, BL], f32, kind="ExternalOutput").ap()
    # DRAM scratch for cross-partition broadcast of B/C (2 alternating)
    bc_scr = [nc.dram_tensor(f"bc_scr{i}", [2 * DS, N], bf16).ap()
              for i in range(2)]

    with tile.TileContext(nc) as tc, ExitStack() as ctx:
        cpool = ctx.enter_context(tc.tile_pool(name="const", bufs=1))
        wpool = ctx.enter_context(tc.tile_pool(name="weights", bufs=2))
        hpool = ctx.enter_context(tc.tile_pool(name="h", bufs=4))
        hbpool = ctx.enter_context(tc.tile_pool(name="hb", bufs=2))
        apool = ctx.enter_context(tc.tile_pool(name="act", bufs=2))
        tpool = ctx.enter_context(tc.tile_pool(name="trans", bufs=2))
        bcpool = ctx.enter_context(tc.tile_pool(name="bc", bufs=1))
        bigpool = ctx.enter_context(tc.tile_pool(name="big", bufs=2))
        psum = ctx.enter_context(tc.tile_pool(name="ps", bufs=8, space="PSUM"))

        def ptile(p, nm="ps"):
            return psum.tile([p, N], f32, padded_shape=[p, 512], name=nm,
                             tag="ps")

        # ---- load constants
        xT = cpool.tile([F, N], f32, tag="xT")
        nc.sync.dma_start(xT, xT_d)
        ipw = cpool.tile([F, DM], f32, tag="ipw")
        nc.sync.dma_start(ipw, ipw_d)
        cls_t = cpool.tile([128, 2 * OUT], f32, tag="cls")
        cls_v = cls_t.rearrange("p (k o) -> p k o", k=2)
        nc.sync.dma_start(cls_v, cls_d.transpose([1, 0, 2]))

        # ---- input projection: h[m] = ipw[:, m*128:...].T @ xT
        h_cur = []
        for m in range(2):
            ps = ptile(128)
            nc.tensor.matmul(ps, ipw[:, ts(m, 128)], xT, start=True, stop=True)
            h0 = hpool.tile([128, N], f32, tag="h")
            nc.scalar.copy(h0, ps)
            h_cur.append(h0)

        for l in range(NL):
            # ---- per-layer weights
            wxb = wpool.tile([128, K * 2 * DI], bf16, tag="wxb")
            wxb_v = wxb.rearrange("p (q j) -> p q j", q=K * 2)
            nc.sync.dma_start(wxb_v, wxb_d[l].transpose([1, 0, 2]))
            wz = wpool.tile([128, 2 * DI], bf16, tag="wz")
            wz_v = wz.rearrange("p (m j) -> p m j", m=2)
            nc.sync.dma_start(wz_v, wz_d[l].transpose([1, 0, 2]))
            xwt = wpool.tile([128, NCH * 64], bf16, tag="xw")
            xwt_v = xwt.rearrange("p (c r) -> p c r", c=NCH)
            nc.sync.dma_start(xwt_v, xw_d[l].transpose([1, 0, 2]))
            dtwt = wpool.tile([DR, DI], bf16, tag="dtw")
            nc.sync.dma_start(dtwt, dtw_d[l])
            owt = wpool.tile([128, NCH * DM], bf16, tag="ow")
            owt_v = owt.rearrange("p (c m) -> p c m", c=NCH)
            nc.sync.dma_start(owt_v, ow_d[l].transpose([1, 0, 2]))
            vt = wpool.tile([128, NCH * 3], f32, tag="vec")
            vt_v = vt.rearrange("p (c k) -> p c k", c=NCH)
            nc.sync.dma_start(vt_v, vec_d[l])

            # ---- bf16 zero-padded h for in_proj (+folded conv taps)
            hp3 = []
            for m in range(2):
                hp = hbpool.tile([128, BL * LP], bf16, tag="hp")
                v = hp.rearrange("p (b t) -> p b t", b=BL)
                nc.vector.memset(v[:, :, 0:K - 1], 0.0)
                nc.scalar.copy(v[:, :, K - 1:],
                               h_cur[m].rearrange("p (b t) -> p b t", b=BL))
                hp3.append(v)

            # ---- in_proj: xb-half with conv folded in (3 prescaled taps),
            # z-half plain.  xc = sum_k (cw_k*Wxb) @ h[t-2+k]
            xz = []
            for j in range(NCH):
                ps = ptile(128)
                ps3 = ps.rearrange("p (b t) -> p b t", b=BL)
                mm = 0
                for k in range(K):
                    for m in range(2):
                        nc.tensor.matmul(ps3, wxb_v[:, k * 2 + m, ts(j, 128)],
                                         hp3[m][:, :, k:k + L],
                                         start=(mm == 0), stop=(mm == 5))
                        mm += 1
                xz.append(ps)
            for j in range(NCH):
                ps = ptile(128)
                ps3 = ps.rearrange("p (b t) -> p b t", b=BL)
                for m in range(2):
                    nc.tensor.matmul(ps3, wz_v[:, m, ts(j, 128)],
                                     hp3[m][:, :, K - 1:],
                                     start=(m == 0), stop=(m == 1))
                xz.append(ps)

            # ---- u = silu(xc + conv_b), sz = silu(z)   (all Silu-table)
            u = apool.tile([128, NCH * N], bf16, tag="u")
            u_v = u.rearrange("p (c n) -> p c n", c=NCH)
            sz = apool.tile([128, NCH * N], bf16, tag="sz")
            sz_v = sz.rearrange("p (c n) -> p c n", c=NCH)
            for c in range(NCH):
                nc.scalar.activation(u_v[:, c, :], xz[c], Act.Silu,
                                     bias=vt_v[:, c, 0:1])
                nc.scalar.activation(sz_v[:, c, :], xz[NCH + c], Act.Silu)

            # ---- x_proj: dbc = xw @ u   (64 x N; rows 0:16 dt, 32:64 B,C;
            # rows 16:32 zero-padded so B,C start on a partition quadrant)
            dbc = psum.tile([64, N], f32, padded_shape=[64, 512], tag="ps")
            for c in range(NCH):
                nc.tensor.matmul(dbc, xwt_v[:, c, :], u_v[:, c, :],
                                 start=(c == 0), stop=(c == NCH - 1))
            dt_sb = tpool.tile([DR, N], bf16, tag="dt", bufs=2)
            nc.scalar.copy(dt_sb, dbc[0:DR, :])
            bc_sb = tpool.tile([2 * DS, N], bf16, tag="bc", bufs=2)
            nc.scalar.copy(bc_sb, dbc[32:64, :])

            # ---- broadcast B,C to all 128 partitions via DRAM roundtrip
            # expanded layout: free = (s, b, t), t innermost for the scan
            scr = bc_scr[l % 2]
            nc.sync.dma_start(scr, bc_sb)
            Brep = bcpool.tile([128, EX], bf16, tag="Brep")
            Crep = bcpool.tile([128, EX], bf16, tag="Crep")
            srcB = scr[0:DS, :].unsqueeze(0).broadcast_to([128, DS, N])
            srcC = scr[DS:2 * DS, :].unsqueeze(0).broadcast_to([128, DS, N])
            nc.sync.dma_start(Brep.rearrange("p (s n) -> p s n", s=DS), srcB)
            nc.sync.dma_start(Crep.rearrange("p (s n) -> p s n", s=DS), srcC)
            Brep4 = Brep.rearrange("p (s b t) -> p s b t", s=DS, b=BL)

            # ---- out_proj accumulators
            op_ps = [ptile(128) for _ in range(2)]

            # ---- delta[c] = softplus(dtw @ dt + dtb) = ln(1 + exp(.)),
            # all chunks up front (no Softplus table on gen3)
            delta = []
            for c in range(NCH):
                dtp = ptile(128)
                nc.tensor.matmul(dtp, dtwt[:, ts(c, 128)], dt_sb,
                                 start=True, stop=True)
                ex = tpool.tile([128, N], f32, tag="ex", bufs=2)
                nc.scalar.activation(ex, dtp, Act.Exp, bias=vt_v[:, c, 1:2])
                dl = tpool.tile([128, N], f32, tag="dl", bufs=4)
                nc.scalar.activation(dl, ex, Act.Ln, bias=1.0)
                delta.append(dl)

            # ---- software-pipelined ssm chunk loop
            Dp, inj, hs = {}, {}, {}

            def S1(c):
                # Dpow[:, s, b, t] = exp(-(s+1)*delta)  (A[d,s] = -(s+1));
                # t=0 zeroed on ACT too (Copy*0) to keep WAW on one engine
                t = bigpool.tile([128, EX], bf16, tag="Dp", name="Dp")
                Dp[c] = t
                t4 = t.rearrange("p (s b t) -> p s b t", s=DS, b=BL)
                for s in range(DS):
                    nc.scalar.activation(t[:, ts(s, N)], delta[c], Act.Exp,
                                         scale=-(s + 1.0))
                nc.scalar.mul(t4[:, :, :, 0:1], t4[:, :, :, 0:1], 0.0)
                du = tpool.tile([128, N], bf16, tag="du", bufs=2)
                nc.gpsimd.tensor_mul(du, delta[c], u_v[:, c, :])
                du4 = (du.rearrange("p (b t) -> p b t", b=BL).unsqueeze(1)
                       .broadcast_to([128, DS, BL, L]))
                ij = bigpool.tile([128, EX], bf16, tag="inj", name="inj")
                inj[c] = ij
                ij4 = ij.rearrange("p (s b t) -> p s b t", s=DS, b=BL)
                nc.vector.tensor_mul(ij4, du4, Brep4)

            def S2(c):
                # scan: hs[t] = Dp[t]*hs[t-1] + inj[t]
                t = bigpool.tile([128, EX], bf16, tag="hs", name="hs")
                hs[c] = t
                nc.vector.tensor_tensor_scan(t, Dp[c], inj[c], 0.0,
                                             Alu.mult, Alu.add)

            def S3(c):
                nc.gpsimd.tensor_mul(hs[c], hs[c], Crep)

            def S4(c):
                # y = sum_s hs*C: tree L1 on DVE, L2..L4 on Pool
                h4 = hs[c].rearrange("p (s b t) -> p s b t", s=DS, b=BL)
                nc.vector.tensor_add(h4[:, 0:8, :, :], h4[:, 0:8, :, :],
                                     h4[:, 8:16, :, :])
                nc.gpsimd.tensor_add(h4[:, 0:4, :, :], h4[:, 0:4, :, :],
                                     h4[:, 4:8, :, :])
                nc.gpsimd.tensor_add(h4[:, 0:2, :, :], h4[:, 0:2, :, :],
                                     h4[:, 2:4, :, :])
                ysum = tpool.tile([128, N], f32, tag="ys", bufs=2)
                y3 = ysum.rearrange("p (b t) -> p b t", b=BL)
                nc.gpsimd.tensor_add(y3, h4[:, 0, :, :], h4[:, 1, :, :])
                # yg = (u*D + ysum) * silu(z)
                yg = tpool.tile([128, N], bf16, tag="yg", bufs=2)
                nc.vector.scalar_tensor_tensor(yg, u_v[:, c, :],
                                               vt_v[:, c, 2:3], ysum,
                                               Alu.mult, Alu.add)
                nc.gpsimd.tensor_mul(yg, yg, sz_v[:, c, :])
                for m in range(2):
                    nc.tensor.matmul(op_ps[m], owt_v[:, c, ts(m, 128)], yg,
                                     start=(c == 0), stop=(c == NCH - 1))

            S1(0)
            S2(0)
            S1(1)
            for c in range(NCH):
                if c + 1 < NCH:
                    S2(c + 1)
                S3(c)
                if c + 2 < NCH:
                    S1(c + 2)
                S4(c)

            # ---- residual
            h_new = []
            for m in range(2):
                hn = hpool.tile([128, N], f32, tag="h")
                nc.vector.tensor_add(hn, h_cur[m], op_ps[m])
                h_new.append(hn)
            h_cur = h_new

        # ---- classifier (mean over t folded into weights)
        lg = psum.tile([OUT, N], f32, padded_shape=[OUT, 512], tag="ps")
        for k in range(2):
            nc.tensor.matmul(lg, cls_v[:, k, :], h_cur[k],
                             start=(k == 0), stop=(k == 1))
        lgm = cpool.tile([OUT, BL], f32, tag="lgm")
        nc.vector.tensor_reduce(lgm, lg.rearrange("p (b t) -> p b t", b=BL),
                                axis=mybir.AxisListType.X, op=Alu.add)
        nc.sync.dma_start(out_d, lgm)

    nc.compile()
    return nc


def _prep_weights(inputs):
    from ml_dtypes import bfloat16 as bf
    f = np.float32
    ipwT = np.ascontiguousarray(inputs["input_proj_w"].T, dtype=f)
    inw = np.asarray(inputs["in_proj_w"], dtype=f)        # (NL, 1024, 256)
    cw = np.asarray(inputs["conv_w"], dtype=f)            # (NL, 512, 3)
    # tap k prescaled: Wk[d, m] = cw[d, k] * Wxb[d, m]
    wxb = inw[:, None, :DI, :] * cw.transpose(0, 2, 1)[:, :, :, None]
    wxbT = np.ascontiguousarray(
        wxb.transpose(0, 1, 3, 2).reshape(NL, K, 2, 128, DI)
        .reshape(NL, K * 2, 128, DI), dtype=bf)
    wzT = np.ascontiguousarray(
        inw[:, DI:, :].transpose(0, 2, 1).reshape(NL, 2, 128, DI), dtype=bf)
    xw_pad = np.zeros((NL, 64, DI), f)
    xw_pad[:, 0:DR] = inputs["x_proj_w"][:, 0:DR]
    xw_pad[:, 32:64] = inputs["x_proj_w"][:, DR:DR + 2 * DS]
    xwT = np.ascontiguousarray(
        xw_pad.transpose(0, 2, 1).reshape(NL, NCH, 128, 64), dtype=bf)
    dtwT = np.ascontiguousarray(
        np.asarray(inputs["dt_proj_w"]).transpose(0, 2, 1), dtype=bf)
    owT = np.ascontiguousarray(
        np.asarray(inputs["out_proj_w"]).transpose(0, 2, 1)
        .reshape(NL, NCH, 128, DM), dtype=bf)
    vec = np.stack([np.asarray(inputs["conv_b"]).reshape(NL, NCH, 128),
                    np.asarray(inputs["dt_proj_b"]).reshape(NL, NCH, 128),
                    np.asarray(inputs["D"]).reshape(NL, NCH, 128)], axis=-1)
    vec = np.ascontiguousarray(vec.transpose(0, 2, 1, 3), dtype=f)
    clsT = np.ascontiguousarray(
        (np.asarray(inputs["classifier_w"]).T / np.float32(L))
        .reshape(2, 128, OUT), dtype=f)
    return {"ipwT": ipwT, "wxbT": wxbT, "wzT": wzT, "xwT": xwT, "dtwT": dtwT,
            "owT": owT, "vec": vec, "clsT": clsT}


def _run(inputs, trace=False):
    from concourse.bass_utils import run_bass_kernel_spmd
    if "nc" not in _CACHE:
        _CACHE["nc"] = _build()
    nc = _CACHE["nc"]
    w = _prep_weights(inputs)
    x = np.asarray(inputs["x"], dtype=np.float32)
    in_maps = []
    for i in range(NCORES):
        xs = x[i * BL:(i + 1) * BL, 0]                 # (16, 28, 28) b,t,f
        xT = np.ascontiguousarray(xs.transpose(2, 0, 1).reshape(F, N))
        m = {"xT": xT}
        m.update(w)
        in_maps.append(m)
    res = run_bass_kernel_spmd(nc, in_maps, list(range(NCORES)), trace=trace)
    parts = [res.results[i]["out"].T for i in range(NCORES)]   # (16, 10) each
    out = np.ascontiguousarray(np.concatenate(parts, axis=0), dtype=np.float32)
    return out, res


def kernel(**inputs) -> np.ndarray:
    out, _ = _run(inputs, trace=False)
    return out
